# revision 36
# baseline (speedup 1.0000x reference)
"""Multi-head attention (B=8, N=1024, C=768, H=8) on 8 Trainium2 NeuronCores.

Sharding: pure data-parallel over batch — core b computes batch element b
end-to-end (no collectives).

Per-core algorithm (working dtype fp16: full PE rate + fast weight load;
fp32 PSUM accumulation everywhere; softmax-sum chain kept in float32r):
  1. x [N,C] -> xT [C,N] via hardware DMA transpose (no PE/DVE cost)
  2. qT/kT per head in padded [128,N] layout (zero weight columns pad head dim
     96->128 so the scores contraction uses K=128), V in natural [N,C] layout
     with a ones-column appended per head (softmax sums come out of the AV
     matmul for free).
  3. S^T = kT_h-slices @ qT_h per (j-tile, i-half): PSUM [128, 512]
     E^T = exp(S^T * hd^-0.5) on ACT (no max subtraction: |scores| <~ 6).
  4. O'^T[h] = sum_jt  V_aug[jt,h]-stationary @ E^T[jt]: PSUM [97, 512] x2,
     row 96 = softmax sums per i.
  5. Late normalization: broadcast 1/sums via K=1 matmul outer(ones, sums),
     reciprocal on DVE, one full-width multiply into the per-head AO^T tile.
  6. y = sum over zero-padded head tiles @ proj_w: natural [N,C] -> DMA out.

Scheduling: single PSUM pool for the whole kernel (no cross-phase stack-pool
barriers); per-head attention is interleaved with the production of the next
head's q/k tiles so ACT exp time hides under PE matmul time.

Bias handling: k-bias provably cancels in softmax; q-bias added at qT eviction
(per-partition); v-bias and proj-bias folded host-side (y += bv @ Wp + bp).
All biases are zero for this problem so those paths are skipped.
"""

import numpy as np

import concourse.bacc as bacc
import concourse.tile as tile
import concourse.mybir as mybir
from concourse import masks
from concourse.bass_utils import run_bass_kernel_spmd

f32 = mybir.dt.float32
f32r = mybir.dt.float32r
bf16 = mybir.dt.bfloat16
AF = mybir.ActivationFunctionType

import os
import ml_dtypes
WDT_MODE = os.environ.get("KERNEL_WDT", "fp16")
WDT = {"bf16": bf16, "fp16": mybir.dt.float16, "f32r": f32r}[WDT_MODE]
WNP = {"bf16": ml_dtypes.bfloat16, "fp16": np.float16, "f32r": np.float32}[WDT_MODE]

B, N, C = 8, 1024, 768
H, HD = 8, 96
NT, CT = N // 128, C // 128  # 8 token tiles, 6 channel tiles
PAD = 128                    # padded per-head dim for q/k
SCALE = float(HD) ** -0.5
VW = HD + 1                  # head block width in V buffer (96 v cols + ones)


def _emit_body(nc, tc, pools, tensors, with_qbias, first=True):
    stage, wstage, epool, npool, ps = pools
    x, wqk, wv, pw, qb, out = tensors["io"]
    ident, ones_f, ones_row = tensors["const"]  # ident/ones_row unused now
    wv_sb, pw_sb, qb_sb = tensors["w"]
    xT, qkT, V_sb, AOT = tensors["buf"]

    # prefetch head-0 q/k weights so the first qk matmul is gated only by
    # the first transpose, not by the whole DMA queue
    wt_pre = {}
    for t in (0, H):
        wt_pre[t] = wstage.tile([128, CT * PAD], WDT, tag="wqk", name=f"wtp{t}")
        nc.sync.dma_start(wt_pre[t][:], wqk[t])

    # ---- Phase A: x -> xT via hardware DMA transpose (fp16 is 2-byte so the
    # xbar path is legal; zero PE/DVE cost) ----
    for ct in range(CT):
        nc.sync.dma_start_transpose(xT[ct][:], x[:, ct * 128:(ct + 1) * 128])

    if first:
        # weight loads issued after x so they don't crowd the DMA queues at t=0
        for k in range(CT):
            nc.sync.dma_start(wv_sb[k][:], wv[k])
        if with_qbias:
            for h in range(H):
                nc.sync.dma_start(qb_sb[h][:], qb[h].rearrange("p -> p 1"))

    def emit_qk(t, wt=None):
        """Produce qkT[t] (padded head tile) into the streaming ring."""
        qkT[t] = wstage.tile([128, N], WDT, tag="qkT", name=f"qkT{t}", bufs=6)
        pst = ps.tile([128, N], f32, tag="qk", name="qkps", bufs=1)
        if wt is None:
            wt = wstage.tile([128, CT * PAD], WDT, tag="wqk", name="wt")
            nc.sync.dma_start(wt[:], wqk[t])
        for k in range(CT):
            for ic in range(2):
                nc.tensor.matmul(
                    pst[:, ic * 512:(ic + 1) * 512],
                    wt[:, k * PAD:(k + 1) * PAD],
                    xT[k][:, ic * 512:(ic + 1) * 512],
                    start=(k == 0), stop=(k == CT - 1),
                )
        if with_qbias and t < H:
            nc.scalar.activation(qkT[t][:], pst[:], AF.Identity, bias=qb_sb[t][:])
        else:
            for ic in range(2):
                nc.vector.tensor_copy(qkT[t][:, ic * 512:(ic + 1) * 512],
                                      pst[:, ic * 512:(ic + 1) * 512])

    def emit_v():
        HB = C // 2  # 384 = 4 head blocks
        for nt in range(NT):
            pv = [ps.tile([128, HB], f32, tag="sc", name=f"vps{half}", bufs=3)
                  for half in range(2)]
            for k in range(CT):
                lhsT = xT[k][:, nt * 128:(nt + 1) * 128]
                for half in range(2):
                    nc.tensor.matmul(pv[half][:], lhsT,
                                     wv_sb[k][:, half * HB:(half + 1) * HB],
                                     start=(k == 0), stop=(k == CT - 1))
            vv = V_sb[nt][:].rearrange("p (h s) -> p h s", h=H)
            for half in range(2):
                nc.vector.tensor_copy(
                    vv[:, half * 4:(half + 1) * 4, 0:HD],
                    pv[half][:].rearrange("p (h d) -> p h d", h=4))
            for h in range(H):
                nc.gpsimd.tensor_copy(
                    V_sb[nt][:, VW * h + HD: VW * h + VW], ones_f[:, 0:1])

    def emit_head(h, after_attn=None):
        qt, kt = qkT[h], qkT[H + h]
        av = [ps.tile([97, 512], f32, tag="av", name=f"av{ic}", bufs=3) for ic in range(2)]

        def emit_av(et_, jt_):
            vh = V_sb[jt_][:, VW * h: VW * h + VW]
            for ic in range(2):
                nc.tensor.matmul(
                    av[ic][:],
                    vh,
                    et_[:, ic * 512:(ic + 1) * 512],
                    start=(jt_ == 0), stop=(jt_ == NT - 1),
                )

        # software pipeline: AV matmuls run one j-tile behind the scores so
        # the in-order PE never waits on ACT's exp latency
        pending = None
        for jt in range(NT):
            et = epool.tile([128, N], WDT, tag="et", name="et")
            for ic in range(2):
                sc = ps.tile([128, 512], f32, tag="sc", name="sc", bufs=3)
                nc.tensor.matmul(
                    sc[:],
                    kt[:, jt * 128:(jt + 1) * 128],
                    qt[:, ic * 512:(ic + 1) * 512],
                    start=True, stop=True,
                )
                nc.scalar.activation(et[:, ic * 512:(ic + 1) * 512], sc[:],
                                     AF.Exp, scale=SCALE)
            if pending is not None:
                emit_av(*pending)
            pending = (et, jt)
        emit_av(*pending)
        if after_attn is not None:
            after_attn()
        for ic in range(2):
            sl = slice(ic * 512, (ic + 1) * 512)
            sums = npool.tile([1, 512], f32r, tag="nrm", name="sums", bufs=3)
            nc.scalar.copy(sums[:], av[ic][96:97, :])
            nb = ps.tile([96, 512], f32, tag="av", name="nb", bufs=3)
            nc.tensor.matmul(nb[:], ones_row[:], sums[:], start=True, stop=True)
            rec = npool.tile([96, 512], f32, tag="nrm", name="rec", bufs=3)
            nc.vector.reciprocal(rec[:], nb[:])
            nc.vector.tensor_mul(AOT[h][0:96, sl], av[ic][0:96, :], rec[:])

    # interleave: q/k for head 0, V, then per-head attention with the next
    # head's q/k production (exp on ACT hides under PE matmuls of B-phase)
    emit_qk(0, wt_pre[0])
    emit_qk(H + 0, wt_pre[H])
    emit_v()
    for h in range(H):
        def _fill(hh=h):
            if hh + 1 < H:
                emit_qk(hh + 1)
                emit_qk(H + hh + 1)
            if hh == 0 and first:
                for j in range(H):
                    nc.sync.dma_start(pw_sb[j][:], pw[j])
        emit_head(h, after_attn=_fill)

    # ---- Phase E: output projection ----
    HB = C // 2
    for it in range(NT):
        yp = [ps.tile([128, HB], f32, tag="sc", name=f"yps{half}", bufs=3)
              for half in range(2)]
        for hh in range(H):
            a = AOT[hh][:, it * 128:(it + 1) * 128]
            for half in range(2):
                nc.tensor.matmul(yp[half][:], a,
                                 pw_sb[hh][:, half * HB:(half + 1) * HB],
                                 start=(hh == 0), stop=(hh == H - 1))
        yst = stage.tile([128, C], f32, tag="ys", name="yst", bufs=2)
        for half in range(2):
            nc.vector.tensor_copy(yst[:, half * HB:(half + 1) * HB], yp[half][:])
        nc.sync.dma_start(out[it * 128:(it + 1) * 128, :], yst[:])


def build_program(with_qbias=False, repeat=1):
    """Build + bacc-compile the single-core SPMD program."""
    nc = bacc.Bacc("TRN2", target_bir_lowering=False)
    x = nc.dram_tensor("x", [N, C], WDT, kind="ExternalInput")
    wqk = nc.dram_tensor("wqk", [2 * H, 128, CT * PAD], WDT, kind="ExternalInput")
    wv = nc.dram_tensor("wv", [CT, 128, C], WDT, kind="ExternalInput")
    pw = nc.dram_tensor("pw", [H, PAD, C], WDT, kind="ExternalInput")
    qb = (nc.dram_tensor("qb", [H, PAD], f32, kind="ExternalInput")
          if with_qbias else None)
    out = nc.dram_tensor("out", [N, C], f32, kind="ExternalOutput")

    with tile.TileContext(nc) as tc:
        with tc.tile_pool(name="const", bufs=1) as constp, \
             tc.tile_pool(name="persist", bufs=1) as persist, \
             tc.tile_pool(name="stage", bufs=3) as stage, \
             tc.tile_pool(name="wstage", bufs=4) as wstage, \
             tc.tile_pool(name="epool", bufs=6) as epool, \
             tc.tile_pool(name="npool", bufs=2) as npool, \
             tc.tile_pool(name="ps", bufs=2, space="PSUM") as ps:

            ones_f = constp.tile([128, HD], f32, tag="ones_f", name="ones_f")
            nc.vector.memset(ones_f[:], 1.0)
            ones_row = constp.tile([1, HD], f32r, tag="ones_row", name="ones_row")
            nc.vector.tensor_copy(ones_row[:], ones_f[0:1, :])

            wv_sb = [persist.tile([128, C], WDT, tag=f"wv{k}", name=f"wv{k}")
                     for k in range(CT)]
            pw_sb = [persist.tile([128, C], WDT, tag=f"pw{h}", name=f"pw{h}")
                     for h in range(H)]
            qb_sb = None
            if with_qbias:
                qb_sb = [persist.tile([128, 1], f32, tag=f"qb{h}", name=f"qb{h}")
                         for h in range(H)]

            xT = [persist.tile([128, N], WDT, tag=f"xT{k}", name=f"xT{k}")
                  for k in range(CT)]
            qkT = {}
            V_sb = [persist.tile([128, VW * H], WDT, tag=f"V{nt}", name=f"V{nt}")
                    for nt in range(NT)]
            AOT = [persist.tile([128, N], WDT, tag=f"AOT{h}", name=f"AOT{h}")
                   for h in range(H)]
            zrow = constp.tile([32, N], f32, tag="zrow", name="zrow")
            nc.vector.memset(zrow[:], 0.0)
            for h in range(H):
                nc.vector.tensor_copy(AOT[h][96:128, :], zrow[:])

            pools = (stage, wstage, epool, npool, ps)
            tensors = {
                "io": (x, wqk, wv, pw, qb, out),
                "const": (None, ones_f, ones_row),
                "w": (wv_sb, pw_sb, qb_sb),
                "buf": (xT, qkT, V_sb, AOT),
            }
            for rep in range(repeat):
                _emit_body(nc, tc, pools, tensors, with_qbias, first=(rep == 0))

    nc.compile()
    return nc


def prepare_host_inputs(x, qkv_w, qkv_b, proj_w, proj_b):
    x = np.ascontiguousarray(np.asarray(x, dtype=np.float32))
    qkv_w = np.asarray(qkv_w, dtype=np.float32)
    qkv_b = np.asarray(qkv_b, dtype=np.float32)
    proj_w = np.asarray(proj_w, dtype=np.float32)
    proj_b = np.asarray(proj_b, dtype=np.float32)

    wq, wk, wv_np = qkv_w[:, 0:C], qkv_w[:, C:2 * C], qkv_w[:, 2 * C:3 * C]
    bq, bv = qkv_b[0:C], qkv_b[2 * C:3 * C]

    wqk_np = np.zeros((2 * H, CT, 128, PAD), WNP)
    for h in range(H):
        wqk_np[h, :, :, 0:HD] = wq[:, h * HD:(h + 1) * HD].reshape(CT, 128, HD)
        wqk_np[H + h, :, :, 0:HD] = wk[:, h * HD:(h + 1) * HD].reshape(CT, 128, HD)
    # [t, c-tile, c-in-tile, d] -> [t, c-in-tile, c-tile*d] so each per-t DMA
    # is one contiguous 128x768 block
    wqk_np = np.ascontiguousarray(
        wqk_np.transpose(0, 2, 1, 3).reshape(2 * H, 128, CT * PAD))
    wv_t = np.ascontiguousarray(wv_np.reshape(CT, 128, C)).astype(WNP)
    pw_t = np.zeros((H, PAD, C), WNP)
    pw_t[:, 0:HD, :] = proj_w.reshape(H, HD, C)

    with_qbias = bool(np.any(bq))
    base = {"wqk": wqk_np, "wv": wv_t, "pw": pw_t}
    if with_qbias:
        qb_np = np.zeros((H, PAD), np.float32)
        for h in range(H):
            qb_np[h, 0:HD] = bq[h * HD:(h + 1) * HD]
        base["qb"] = qb_np

    # v-bias and proj-bias commute past attention/proj -> host-side add
    post_add = bv @ proj_w + proj_b
    in_maps = [dict(base, x=np.ascontiguousarray(x[b]).astype(WNP)) for b in range(B)]
    return in_maps, with_qbias, post_add


def kernel(x, qkv_w, qkv_b, proj_w, proj_b):
    in_maps, with_qbias, post_add = prepare_host_inputs(
        x, qkv_w, qkv_b, proj_w, proj_b)
    nc = build_program(with_qbias=with_qbias)
    res = run_bass_kernel_spmd(nc, in_maps, core_ids=list(range(B)))
    y = np.stack([res.results[b]["out"] for b in range(B)], axis=0)
    if np.any(post_add):
        y = y + post_add[None, None, :].astype(np.float32)
    return np.ascontiguousarray(y.astype(np.float32))
